# revision 7
# baseline (speedup 1.0000x reference)
"""Trainium2 Bass/Tile kernel for an RNN-T Joiner:

    enc_p = encoder_out @ W_enc.T + b_enc          (N,200,512)
    dec_p = decoder_out @ W_dec.T + b_dec          (N,50,512)
    act   = tanh(enc_p[:,:,None,:] + dec_p[:,None,:,:])
    out   = act @ W_out.T + b_out                  (N,200,50,500)

Sharding: data-parallel over N=8 — core i computes batch element i end to
end; weights are replicated to every core. All device inputs are staged
host-side in the PE-friendly layout: contraction dim leading (pre-
transposed) and bf16 — the standard inference-deployment format. Biases
stay fp32; the output is fp32.

Per-core dataflow (everything on-chip after the initial loads):
  - DMA encT/decT/W_encT/W_decT/W_outT straight into SBUF (no on-device
    transposes at all),
  - project:  enc_pT[j,t], dec_pT[j,u]  (PE bf16 -> fp32 PSUM, bias folded
    in via the ACT copy out of PSUM, stored bf16),
  - per 64-t chunk: broadcast-add (bf16, 0-stride APs) + in-place tanh
    (ACT) builds actT[j, cell] (cell = t*U+u). Chunk 0 runs its adds on
    the DVE in halves so the first vocab matmuls start within ~2us;
    steady-state chunks put 1 add on DVE + 3 on GPSIMD, because the DVE
    also owns every PSUM drain (GPSIMD cannot address PSUM),
  - vocab matmul per 128-cell block: psum[cell,v] = sum_jb actT_blk.T @
    W_outT[jb]  (bf16 -> fast weight load; one long back-to-back MM
    stream keeps the HAM clock at full rate; scratch keep-warm matmuls
    bridge the projections->first-chunk gap),
  - +b_out fused into the PSUM->SBUF drain (DVE tensor_tensor with a
    pre-broadcast fp32 bias tile), output DMA in ~1.25MB batches.
"""

import numpy as np
from contextlib import ExitStack

N, T, U = 8, 200, 50
E = J = 512
V = 500
CELLS = T * U
P = 128
KB = J // P  # 4 contraction blocks

_NC_CACHE = {}


def _build_nc():
    import concourse.mybir as mybir
    import concourse.tile as tile
    from concourse import bacc

    f32 = mybir.dt.float32
    bf16 = mybir.dt.bfloat16
    ADD = mybir.AluOpType.add
    TANH = mybir.ActivationFunctionType.Tanh
    IDENT = mybir.ActivationFunctionType.Identity

    nc = bacc.Bacc("TRN2", target_bir_lowering=False, debug=False)

    encT_d = nc.dram_tensor("encT", [E, T], bf16, kind="ExternalInput").ap()
    decT_d = nc.dram_tensor("decT", [E, U], bf16, kind="ExternalInput").ap()
    wencT_d = nc.dram_tensor("W_encT", [E, J], bf16, kind="ExternalInput").ap()
    benc_d = nc.dram_tensor("b_enc", [J], f32, kind="ExternalInput").ap()
    wdecT_d = nc.dram_tensor("W_decT", [E, J], bf16, kind="ExternalInput").ap()
    bdec_d = nc.dram_tensor("b_dec", [J], f32, kind="ExternalInput").ap()
    woutT_d = nc.dram_tensor("W_outT", [J, V], bf16, kind="ExternalInput").ap()
    bout_d = nc.dram_tensor("b_out", [V], f32, kind="ExternalInput").ap()
    out_d = nc.dram_tensor("logits", [CELLS, V], f32, kind="ExternalOutput").ap()

    with tile.TileContext(nc) as tc, ExitStack() as ctx:
        const = ctx.enter_context(tc.tile_pool(name="const", bufs=1))
        pj_ps = ctx.enter_context(tc.tile_pool(name="pj_ps", bufs=2, space="PSUM"))
        mm_ps = ctx.enter_context(tc.tile_pool(name="mm_ps", bufs=6, space="PSUM"))
        act_pool = ctx.enter_context(tc.tile_pool(name="act", bufs=2))
        out_pool = ctx.enter_context(tc.tile_pool(name="outp", bufs=3))

        def load_rows(dram_ap, cols, name):
            tiles = []
            for kb in range(KB):
                t = const.tile([P, cols], bf16, name=f"{name}{kb}")
                nc.sync.dma_start(t[:], dram_ap[kb * P : (kb + 1) * P, :])
                tiles.append(t)
            return tiles

        # Emission order = scheduler priority: the projection operands gate
        # everything downstream, then W_outT (needed by the first vocab
        # matmul ~10us in), then the small biases.
        encT = load_rows(encT_d, T, "enc")      # 4 x [128(e), 200(t)]
        decT = load_rows(decT_d, U, "dec")      # 4 x [128(e), 50(u)]
        W_encT = load_rows(wencT_d, J, "wenc")  # 4 x [128(e), 512(j)]
        W_decT = load_rows(wdecT_d, J, "wdec")  # 4 x [128(e), 512(j)]
        W_outT = load_rows(woutT_d, V, "wout")  # 4 x [128(j), 500(v)]

        b_enc_sb = const.tile([P, KB], f32)
        nc.sync.dma_start(b_enc_sb[:], benc_d.rearrange("(kb p) -> p kb", p=P))
        b_dec_sb = const.tile([P, KB], f32)
        nc.sync.dma_start(b_dec_sb[:], bdec_d.rearrange("(kb p) -> p kb", p=P))

        # Projections -> enc_pT[jb]: [128(j), T] bf16, dec_pT[jb]: [128(j), U]
        def project(WT, srcT, b_sb, width, nm):
            outs = []
            for jb in range(KB):
                pp = pj_ps.tile([P, T], f32, tag="pj", name=f"{nm}_ps{jb}")
                for kb in range(KB):
                    nc.tensor.matmul(
                        pp[:, :width],
                        lhsT=WT[kb][:, jb * P : (jb + 1) * P],
                        rhs=srcT[kb][:, :width],
                        start=(kb == 0),
                        stop=(kb == KB - 1),
                    )
                o = const.tile([P, width], bf16, name=f"{nm}{jb}")
                nc.scalar.activation(o[:], pp[:, :width], IDENT, bias=b_sb[:, jb : jb + 1])
                outs.append(o)
            return outs

        enc_pT = project(W_encT, encT, b_enc_sb, T, "encp")
        dec_pT = project(W_decT, decT, b_dec_sb, U, "decp")

        # b_out broadcast to all 128 partitions via a K=1 ones matmul
        bout_row = const.tile([1, V], f32)
        nc.sync.dma_start(bout_row[:], bout_d[None, :])
        ones_col = const.tile([1, P], f32)
        nc.gpsimd.memset(ones_col[:], 1.0)
        bp = mm_ps.tile([P, V], f32, tag="mm")
        nc.tensor.matmul(bp[:], lhsT=ones_col[:], rhs=bout_row[:], start=True, stop=True)
        bout_rep = const.tile([P, V], f32)
        nc.vector.tensor_copy(bout_rep[:], bp[:])

        def keep_warm(i):
            # Scratch matmul, result never read: holds the PE busy across
            # the projections -> first-chunk-acts gap so the HAM clock
            # doesn't re-throttle (idle >3.4us drops PE to half rate).
            kw = pj_ps.tile([P, T], f32, tag="pj", name=f"warm{i}")
            nc.tensor.matmul(
                kw[:, :P], lhsT=W_decT[0][:, :P], rhs=W_encT[0][:, :P],
                start=True, stop=True,
            )

        # Main loop: cell = t*U+u, t-chunks of 64 (64*50 = 3200 = 25*128)
        CHUNKS = [(0, 64), (64, 64), (128, 64), (192, 8)]
        ACT_COLS = 64 * U
        BATCH = 5  # output blocks per DMA (5*128 cells * 2000B = 1.28 MB)

        def gen_acts(ci, t0, L):
            C = L * U
            acts = []
            for jb in range(KB):
                s = act_pool.tile([P, ACT_COLS], bf16, tag=f"act{jb}", name=f"s{ci}_{jb}")
                # Chunk 0 is the ramp: all adds on the DVE (it has no drains
                # yet), emitted in halves with tanh chasing each half so the
                # first vocab matmuls start ~2us in. Steady-state chunks:
                # 1 DVE add + 3 GPSIMD adds per chunk (DVE owns the drains).
                if ci == 0:
                    add_eng, n_parts = nc.vector, 2
                else:
                    add_eng = nc.vector if jb == 0 else nc.gpsimd
                    n_parts = 2
                tl = L // n_parts
                for q in range(n_parts):
                    qt = t0 + q * tl
                    qc = q * tl * U
                    add_eng.tensor_tensor(
                        out=s[:, qc : qc + tl * U].rearrange("p (l u) -> p l u", u=U),
                        in0=dec_pT[jb][:, None, :].broadcast_to([P, tl, U]),
                        in1=enc_pT[jb][:, qt : qt + tl][:, :, None].broadcast_to(
                            [P, tl, U]
                        ),
                        op=ADD,
                    )
                    h = tl * U // 2
                    nc.scalar.activation(s[:, qc : qc + h], s[:, qc : qc + h], TANH)
                    nc.scalar.activation(
                        s[:, qc + h : qc + tl * U], s[:, qc + h : qc + tl * U], TANH
                    )
                if ci == 0 and jb < 3:
                    keep_warm(jb)
                acts.append(s)
            return acts

        for ci, (t0, L) in enumerate(CHUNKS):
            C = L * U
            c_base = t0 * U
            acts = gen_acts(ci, t0, L)

            nfull = C // P
            tail = C % P
            b0 = 0
            while b0 < nfull:
                nb = min(BATCH, nfull - b0)
                ob = out_pool.tile([P, BATCH * V], f32, tag="ob", name=f"ob{ci}_{b0}")
                for q in range(nb):
                    blk = b0 + q
                    ps = mm_ps.tile([P, V], f32, tag="mm", name=f"ps{ci}_{blk}")
                    for jb in range(KB):
                        nc.tensor.matmul(
                            ps[:],
                            lhsT=acts[jb][:, blk * P : (blk + 1) * P],
                            rhs=W_outT[jb][:],
                            start=(jb == 0),
                            stop=(jb == KB - 1),
                        )
                    nc.vector.tensor_tensor(
                        out=ob[:, q * V : (q + 1) * V], in0=ps[:], in1=bout_rep[:], op=ADD
                    )
                c0 = c_base + b0 * P
                dst = out_d[c0 : c0 + nb * P, :].rearrange("(b p) v -> p b v", p=P)
                nc.sync.dma_start(dst, ob[:, : nb * V].rearrange("p (b v) -> p b v", v=V))
                b0 += nb
            if tail:
                ps = mm_ps.tile([P, V], f32, tag="mm", name=f"ps{ci}_t")
                for jb in range(KB):
                    nc.tensor.matmul(
                        ps[:tail, :],
                        lhsT=acts[jb][:, nfull * P : nfull * P + tail],
                        rhs=W_outT[jb][:],
                        start=(jb == 0),
                        stop=(jb == KB - 1),
                    )
                obt = out_pool.tile([P, BATCH * V], f32, tag="ob", name=f"obt{ci}")
                nc.vector.tensor_tensor(
                    out=obt[:tail, :V], in0=ps[:tail, :], in1=bout_rep[:tail, :], op=ADD
                )
                c0 = c_base + nfull * P
                nc.sync.dma_start(out_d[c0 : c0 + tail, :], obt[:tail, :V])

    nc.compile()
    return nc


def get_nc():
    if "nc" not in _NC_CACHE:
        _NC_CACHE["nc"] = _build_nc()
    return _NC_CACHE["nc"]


def make_in_maps(inputs):
    import ml_dtypes

    bf16 = ml_dtypes.bfloat16

    def t_bf16(a):  # host-side: transpose + cast = accelerator staging layout
        return np.ascontiguousarray(np.asarray(a, dtype=np.float32).T).astype(bf16)

    enc = np.asarray(inputs["encoder_out"], dtype=np.float32)
    dec = np.asarray(inputs["decoder_out"], dtype=np.float32)
    shared = {
        "W_encT": t_bf16(inputs["W_enc"]),
        "W_decT": t_bf16(inputs["W_dec"]),
        "W_outT": t_bf16(inputs["W_out"]),
        "b_enc": np.ascontiguousarray(np.asarray(inputs["b_enc"], dtype=np.float32)),
        "b_dec": np.ascontiguousarray(np.asarray(inputs["b_dec"], dtype=np.float32)),
        "b_out": np.ascontiguousarray(np.asarray(inputs["b_out"], dtype=np.float32)),
    }
    return [
        {"encT": t_bf16(enc[i]), "decT": t_bf16(dec[i]), **shared} for i in range(N)
    ]


def kernel(**inputs):
    from concourse.bass_utils import run_bass_kernel_spmd

    nc = get_nc()
    in_maps = make_in_maps(inputs)
    res = run_bass_kernel_spmd(nc, in_maps, core_ids=list(range(N)))
    out = np.stack([r["logits"] for r in res.results], axis=0)
    return out.reshape(N, T, U, V)


# revision 8
# speedup vs baseline: 1.0513x; 1.0513x over previous
"""Trainium2 Bass/Tile kernel for an RNN-T Joiner:

    enc_p = encoder_out @ W_enc.T + b_enc          (N,200,512)
    dec_p = decoder_out @ W_dec.T + b_dec          (N,50,512)
    act   = tanh(enc_p[:,:,None,:] + dec_p[:,None,:,:])
    out   = act @ W_out.T + b_out                  (N,200,50,500)

Sharding: data-parallel over N=8 — core i computes batch element i end to
end; weights are replicated to every core. All device inputs are staged
host-side in the PE-friendly layout: contraction dim leading (pre-
transposed) and bf16 — the standard inference-deployment format. Biases
stay fp32; the output is fp32.

Per-core dataflow (everything on-chip after the initial loads):
  - DMA encT/decT/W_encT/W_decT/W_outT straight into SBUF (no on-device
    transposes at all),
  - project:  enc_pT[j,t], dec_pT[j,u]  (PE bf16 -> fp32 PSUM, bias folded
    in via the ACT copy out of PSUM, stored bf16),
  - per 64-t chunk: broadcast-add (bf16, 0-stride APs) + in-place tanh
    (ACT) builds actT[j, cell] (cell = t*U+u). Chunk 0 runs its adds on
    the DVE in halves so the first vocab matmuls start within ~2us;
    steady-state chunks put 1 add on DVE + 3 on GPSIMD, because the DVE
    also owns every PSUM drain (GPSIMD cannot address PSUM),
  - vocab matmul per 128-cell block: psum[cell,v] = sum_jb actT_blk.T @
    W_outT[jb]  (bf16 -> fast weight load; one long back-to-back MM
    stream keeps the HAM clock at full rate; scratch keep-warm matmuls
    bridge the projections->first-chunk gap),
  - +b_out fused into the PSUM->SBUF drain (DVE tensor_tensor with a
    pre-broadcast fp32 bias tile), output DMA in ~1.25MB batches.
"""

import numpy as np
from contextlib import ExitStack

N, T, U = 8, 200, 50
E = J = 512
V = 500
CELLS = T * U
P = 128
KB = J // P  # 4 contraction blocks

_NC_CACHE = {}


def _build_nc():
    import concourse.mybir as mybir
    import concourse.tile as tile
    from concourse import bacc

    f32 = mybir.dt.float32
    bf16 = mybir.dt.bfloat16
    ADD = mybir.AluOpType.add
    TANH = mybir.ActivationFunctionType.Tanh
    IDENT = mybir.ActivationFunctionType.Identity

    nc = bacc.Bacc("TRN2", target_bir_lowering=False, debug=False)

    encT_d = nc.dram_tensor("encT", [E, T], bf16, kind="ExternalInput").ap()
    decT_d = nc.dram_tensor("decT", [E, U], bf16, kind="ExternalInput").ap()
    wencT_d = nc.dram_tensor("W_encT", [E, J], bf16, kind="ExternalInput").ap()
    benc_d = nc.dram_tensor("b_enc", [J], f32, kind="ExternalInput").ap()
    wdecT_d = nc.dram_tensor("W_decT", [E, J], bf16, kind="ExternalInput").ap()
    bdec_d = nc.dram_tensor("b_dec", [J], f32, kind="ExternalInput").ap()
    woutT_d = nc.dram_tensor("W_outT", [J, V], bf16, kind="ExternalInput").ap()
    bout_d = nc.dram_tensor("b_out", [V], f32, kind="ExternalInput").ap()
    out_d = nc.dram_tensor("logits", [CELLS, V], f32, kind="ExternalOutput").ap()

    with tile.TileContext(nc) as tc, ExitStack() as ctx:
        const = ctx.enter_context(tc.tile_pool(name="const", bufs=1))
        pj_ps = ctx.enter_context(tc.tile_pool(name="pj_ps", bufs=2, space="PSUM"))
        mm_ps = ctx.enter_context(tc.tile_pool(name="mm_ps", bufs=6, space="PSUM"))
        act_pool = ctx.enter_context(tc.tile_pool(name="act", bufs=2))
        out_pool = ctx.enter_context(tc.tile_pool(name="outp", bufs=3))

        def load_rows(dram_ap, cols, name):
            # One DMA per tensor (21 small dma_starts serialized the
            # prologue to ~25us; consolidated loads finish in ~10us).
            big = const.tile([P, KB * cols], bf16, name=f"{name}_all")
            nc.sync.dma_start(
                big[:].rearrange("p (kb c) -> p kb c", kb=KB),
                dram_ap.rearrange("(kb p) c -> p kb c", p=P),
            )
            return [big[:, kb * cols : (kb + 1) * cols] for kb in range(KB)]

        # Emission order = scheduler priority: the projection operands gate
        # everything downstream, then W_outT (needed by the first vocab
        # matmul ~10us in), then the small biases.
        encT = load_rows(encT_d, T, "enc")      # 4 x [128(e), 200(t)]
        decT = load_rows(decT_d, U, "dec")      # 4 x [128(e), 50(u)]
        W_encT = load_rows(wencT_d, J, "wenc")  # 4 x [128(e), 512(j)]
        W_decT = load_rows(wdecT_d, J, "wdec")  # 4 x [128(e), 512(j)]
        W_outT = load_rows(woutT_d, V, "wout")  # 4 x [128(j), 500(v)]

        b_enc_sb = const.tile([P, KB], f32)
        nc.sync.dma_start(b_enc_sb[:], benc_d.rearrange("(kb p) -> p kb", p=P))
        b_dec_sb = const.tile([P, KB], f32)
        nc.sync.dma_start(b_dec_sb[:], bdec_d.rearrange("(kb p) -> p kb", p=P))

        # Projections -> enc_pT[jb]: [128(j), T] bf16, dec_pT[jb]: [128(j), U]
        def project(WT, srcT, b_sb, width, nm):
            outs = []
            for jb in range(KB):
                pp = pj_ps.tile([P, T], f32, tag="pj", name=f"{nm}_ps{jb}")
                for kb in range(KB):
                    nc.tensor.matmul(
                        pp[:, :width],
                        lhsT=WT[kb][:, jb * P : (jb + 1) * P],
                        rhs=srcT[kb][:, :width],
                        start=(kb == 0),
                        stop=(kb == KB - 1),
                    )
                o = const.tile([P, width], bf16, name=f"{nm}{jb}")
                nc.scalar.activation(o[:], pp[:, :width], IDENT, bias=b_sb[:, jb : jb + 1])
                outs.append(o)
            return outs

        enc_pT = project(W_encT, encT, b_enc_sb, T, "encp")
        dec_pT = project(W_decT, decT, b_dec_sb, U, "decp")

        # b_out broadcast to all 128 partitions via a K=1 ones matmul
        bout_row = const.tile([1, V], f32)
        nc.sync.dma_start(bout_row[:], bout_d[None, :])
        ones_col = const.tile([1, P], f32)
        nc.gpsimd.memset(ones_col[:], 1.0)
        bp = mm_ps.tile([P, V], f32, tag="mm")
        nc.tensor.matmul(bp[:], lhsT=ones_col[:], rhs=bout_row[:], start=True, stop=True)
        bout_rep = const.tile([P, V], f32)
        nc.vector.tensor_copy(bout_rep[:], bp[:])

        def keep_warm(i):
            # Scratch matmul, result never read: holds the PE busy across
            # the projections -> first-chunk-acts gap so the HAM clock
            # doesn't re-throttle (idle >3.4us drops PE to half rate).
            kw = pj_ps.tile([P, T], f32, tag="pj", name=f"warm{i}")
            nc.tensor.matmul(
                kw[:, :P], lhsT=W_decT[0][:, :P], rhs=W_encT[0][:, :P],
                start=True, stop=True,
            )

        # Main loop: cell = t*U+u, t-chunks of 64 (64*50 = 3200 = 25*128)
        CHUNKS = [(0, 64), (64, 64), (128, 64), (192, 8)]
        ACT_COLS = 64 * U
        BATCH = 5  # output blocks per DMA (5*128 cells * 2000B = 1.28 MB)

        def gen_acts(ci, t0, L):
            C = L * U
            acts = []
            for jb in range(KB):
                s = act_pool.tile([P, ACT_COLS], bf16, tag=f"act{jb}", name=f"s{ci}_{jb}")
                # Chunk 0 is the ramp: all adds on the DVE (it has no drains
                # yet), emitted in halves with tanh chasing each half so the
                # first vocab matmuls start ~2us in. Steady-state chunks:
                # 1 DVE add + 3 GPSIMD adds per chunk (DVE owns the drains).
                if ci == 0:
                    add_eng, n_parts = nc.vector, 2
                else:
                    add_eng = nc.vector if jb == 0 else nc.gpsimd
                    n_parts = 2
                tl = L // n_parts
                for q in range(n_parts):
                    qt = t0 + q * tl
                    qc = q * tl * U
                    add_eng.tensor_tensor(
                        out=s[:, qc : qc + tl * U].rearrange("p (l u) -> p l u", u=U),
                        in0=dec_pT[jb][:, None, :].broadcast_to([P, tl, U]),
                        in1=enc_pT[jb][:, qt : qt + tl][:, :, None].broadcast_to(
                            [P, tl, U]
                        ),
                        op=ADD,
                    )
                    h = tl * U // 2
                    nc.scalar.activation(s[:, qc : qc + h], s[:, qc : qc + h], TANH)
                    nc.scalar.activation(
                        s[:, qc + h : qc + tl * U], s[:, qc + h : qc + tl * U], TANH
                    )
                if ci == 0 and jb < 3:
                    keep_warm(jb)
                acts.append(s)
            return acts

        for ci, (t0, L) in enumerate(CHUNKS):
            C = L * U
            c_base = t0 * U
            acts = gen_acts(ci, t0, L)

            nfull = C // P
            tail = C % P
            b0 = 0
            while b0 < nfull:
                nb = min(BATCH, nfull - b0)
                ob = out_pool.tile([P, BATCH * V], f32, tag="ob", name=f"ob{ci}_{b0}")
                for q in range(nb):
                    blk = b0 + q
                    ps = mm_ps.tile([P, V], f32, tag="mm", name=f"ps{ci}_{blk}")
                    for jb in range(KB):
                        nc.tensor.matmul(
                            ps[:],
                            lhsT=acts[jb][:, blk * P : (blk + 1) * P],
                            rhs=W_outT[jb][:],
                            start=(jb == 0),
                            stop=(jb == KB - 1),
                        )
                    nc.vector.tensor_tensor(
                        out=ob[:, q * V : (q + 1) * V], in0=ps[:], in1=bout_rep[:], op=ADD
                    )
                c0 = c_base + b0 * P
                dst = out_d[c0 : c0 + nb * P, :].rearrange("(b p) v -> p b v", p=P)
                nc.sync.dma_start(dst, ob[:, : nb * V].rearrange("p (b v) -> p b v", v=V))
                b0 += nb
            if tail:
                ps = mm_ps.tile([P, V], f32, tag="mm", name=f"ps{ci}_t")
                for jb in range(KB):
                    nc.tensor.matmul(
                        ps[:tail, :],
                        lhsT=acts[jb][:, nfull * P : nfull * P + tail],
                        rhs=W_outT[jb][:],
                        start=(jb == 0),
                        stop=(jb == KB - 1),
                    )
                obt = out_pool.tile([P, BATCH * V], f32, tag="ob", name=f"obt{ci}")
                nc.vector.tensor_tensor(
                    out=obt[:tail, :V], in0=ps[:tail, :], in1=bout_rep[:tail, :], op=ADD
                )
                c0 = c_base + nfull * P
                nc.sync.dma_start(out_d[c0 : c0 + tail, :], obt[:tail, :V])

    nc.compile()
    return nc


def get_nc():
    if "nc" not in _NC_CACHE:
        _NC_CACHE["nc"] = _build_nc()
    return _NC_CACHE["nc"]


def make_in_maps(inputs):
    import ml_dtypes

    bf16 = ml_dtypes.bfloat16

    def t_bf16(a):  # host-side: transpose + cast = accelerator staging layout
        return np.ascontiguousarray(np.asarray(a, dtype=np.float32).T).astype(bf16)

    enc = np.asarray(inputs["encoder_out"], dtype=np.float32)
    dec = np.asarray(inputs["decoder_out"], dtype=np.float32)
    shared = {
        "W_encT": t_bf16(inputs["W_enc"]),
        "W_decT": t_bf16(inputs["W_dec"]),
        "W_outT": t_bf16(inputs["W_out"]),
        "b_enc": np.ascontiguousarray(np.asarray(inputs["b_enc"], dtype=np.float32)),
        "b_dec": np.ascontiguousarray(np.asarray(inputs["b_dec"], dtype=np.float32)),
        "b_out": np.ascontiguousarray(np.asarray(inputs["b_out"], dtype=np.float32)),
    }
    return [
        {"encT": t_bf16(enc[i]), "decT": t_bf16(dec[i]), **shared} for i in range(N)
    ]


def kernel(**inputs):
    from concourse.bass_utils import run_bass_kernel_spmd

    nc = get_nc()
    in_maps = make_in_maps(inputs)
    res = run_bass_kernel_spmd(nc, in_maps, core_ids=list(range(N)))
    out = np.stack([r["logits"] for r in res.results], axis=0)
    return out.reshape(N, T, U, V)
